# revision 15
# baseline (speedup 1.0000x reference)
"""Bass/Tile kernel for nn_MicrotubuleAttention on 8 Trainium2 NeuronCores.

Math: the reference adds (1 - gtp) * NEG (NEG = -1e9) to every causal
off-diagonal score. With gamma >= 1e-3 the smallest penalty is
-1e9 * (1 - exp(-1e-3)) ~= -1e6, so after float32 softmax (max-subtract +
exp) every off-diagonal weight underflows to exactly 0 and attention is
exactly the identity. Hence:

    out = repeat_gqa(x @ Wv) @ Wo = (x @ Wv) @ Wo_folded

where Wo_folded[c*64+d, :] = sum_r Wo[(4c+r)*64+d, :] sums the 4 query-head
row blocks that share KV head c. Q/K/RoPE/polarity/gamma provably do not
affect the f32 output (verified ~1e-6 max rel err against the jax
reference; this fp16 kernel measures 5.1e-4 vs the 2e-2 gate).

Precision: inputs are rounded to fp16 on the host (layout prep + cast is
host-side sharding work; every matmul FLOP stays on device). PSUM
accumulates in fp32; v and out are stored fp16.

Sharding: data parallel over rows, 512 rows/core; Wv/WoF broadcast.
  stage 1:  vT[j, m] = sum_d Wv[d, j] * xT[d, m]     (16 MMs, N=512)
  stage 2:  out[m, n] = sum_j vT[j, m] * WoF[j, n]   (16 MMs, N=512)

Schedule notes (from trace analysis):
 - dma_start costs ~605ns of issuing-sequencer time (128 descriptors);
   every tensor is host-prepped to [128 partitions x one contiguous run]
   so each load is a single instruction at full SDMA line rate.
 - SDMA engines round-robin rings at whole-transfer granularity, so ALL
   loads go on the Sync ring in dependency order (wv -> xt chunks -> wof);
   a second ring would let wof jump ahead of the x stream.
 - Stores also issue from Sync (idle after loads); copies split DVE/ACT.
 - PE HAM: ~14 N=512 warm matmuls bridge t=0 to first real MM so stage 1
   runs at 2.4GHz; the stream then stays dense through stage 2.
 - o_pool/psum bufs sized so no copy ever waits on a store's HBM receipt.
"""

import os
import sys

import numpy as np

for _p in ("/opt/trn_rl_repo", "/opt/pypackages"):
    if os.path.isdir(_p) and _p not in sys.path:
        sys.path.append(_p)

B, T, D_MODEL = 2, 2048, 1024
N_CORES = 8
M_TOTAL = B * T              # 4096 rows
M_CORE = M_TOTAL // N_CORES  # 512 rows per core
P = 128
KK = D_MODEL // P            # 8 contraction chunks of 128
NKV = 256                    # H_KV * D_HEAD
JO = NKV // P                # 2 j-chunks
MC = M_CORE // P             # 4 m-chunks of 128
NH = 2                       # output n-halves of 512

TRACE = False          # test.py flips this to profile
TRACE_CORES = None
LAST_RESULTS = None    # BassKernelResults of the most recent run

_nc_cache = None


def _build_bass():
    import concourse.bass as bass
    import concourse.mybir as mybir
    import concourse.tile as tile
    from concourse import bacc

    f32 = mybir.dt.float32
    f16 = mybir.dt.float16
    ts = bass.ts

    nc = bacc.Bacc(None)
    # host-prepped layouts (see kernel()):
    #  xt : [128, 8*512]   row p, col ko*512+m  = x.T[ko*128+p, m]
    #  wv : [128, 8*256]   row p, col ko*256+j  = Wv[ko*128+p, j]
    #  wof: [128, 2*2*512] row p, col (h*2+jo)*512+n = WoF[jo*128+p, h*512+n]
    #  out: [512, 1024]    row m, col n
    xt_d = nc.declare_dram_parameter("xt", [P, KK * M_CORE], f16, isOutput=False)
    wv_d = nc.declare_dram_parameter("wv", [P, KK * NKV], f16, isOutput=False)
    wof_d = nc.declare_dram_parameter("wof", [P, NH * JO * 512], f16, isOutput=False)
    out_d = nc.declare_dram_parameter("out", [M_CORE, D_MODEL], f16, isOutput=True)

    with tile.TileContext(nc) as tc:
        with (
            tc.tile_pool(name="const", bufs=1) as const,
            tc.tile_pool(name="psum_s1", bufs=2, space="PSUM") as psum_s1,
            tc.tile_pool(name="psum_s2", bufs=3, space="PSUM") as psum_s2,
            tc.tile_pool(name="o_pool", bufs=4) as o_pool,
        ):
            # --- PE warm-keepers (see module docstring) ---
            zeros = const.tile([P, M_CORE], f16)
            nc.vector.memset(zeros, 0.0)
            # tiny dummy activation: forces the 1.3us ACT_TABLE_LOAD into the
            # dead window at t~0 instead of right before the first real copy
            act_warm = const.tile([P, 1], f16)
            nc.scalar.copy(act_warm[:], zeros[:, :1])
            warm = psum_s1.tile([P, M_CORE], f32, tag="s1")
            # dense bridge from t~0 to stage-1 readiness (~4.5us): 8 big MMs
            # then small ones for fine granularity — any PE idle gap resets
            # the HAM busy window and stage 1 runs at 1.2GHz instead of 2.4
            for _ in range(8):
                nc.tensor.matmul(warm[:], lhsT=zeros[:, :P], rhs=zeros[:])
            for _ in range(8):
                nc.tensor.matmul(warm[:, :P], lhsT=zeros[:, :P], rhs=zeros[:, :P])

            wv_sb = const.tile([P, KK, NKV], f16)
            xt_sb = const.tile([P, KK, M_CORE], f16)
            vt_sb = const.tile([P, JO, M_CORE], f16)
            wof_sb = const.tile([P, NH, JO, 512], f16)

            # --- loads: one ring (Sync), in consumption order ---
            nc.sync.dma_start(
                wv_sb[:], wv_d.rearrange("p (ko j) -> p ko j", ko=KK)
            )
            for g in range(4):
                nc.sync.dma_start(
                    xt_sb[:, ts(g, 2), :],
                    xt_d[:, ts(g, 2 * M_CORE)].rearrange("p (k m) -> p k m", k=2),
                )
            for h in range(NH):
                nc.sync.dma_start(
                    wof_sb[:, h, :, :],
                    wof_d[:, ts(h, JO * 512)].rearrange("p (jo n) -> p jo n", jo=JO),
                )

            # --- stage 1: vT[jo][j, m] += Wv[ko, jo]^T-block @ xT[ko] ---
            vt_ps0 = psum_s1.tile([P, M_CORE], f32, tag="s1")
            vt_ps1 = psum_s1.tile([P, M_CORE], f32, tag="s1")
            vt_ps = [vt_ps0, vt_ps1]
            for ko in range(KK):
                for jo in range(JO):
                    nc.tensor.matmul(
                        vt_ps[jo][:],
                        lhsT=wv_sb[:, ko, ts(jo, P)],
                        rhs=xt_sb[:, ko, :],
                        start=(ko == 0),
                        stop=(ko == KK - 1),
                    )
            # m-half split across DVE+ACT: stage 2's first MMs (mi 0/1) only
            # need the low-m half, so they unblock after the first short copy
            nc.vector.tensor_copy(vt_sb[:, 0, :256], vt_ps0[:, :256])
            nc.scalar.copy(vt_sb[:, 0, 256:], vt_ps0[:, 256:])
            nc.vector.tensor_copy(vt_sb[:, 1, :256], vt_ps1[:, :256])
            nc.scalar.copy(vt_sb[:, 1, 256:], vt_ps1[:, 256:])

            # --- stage 2 + drain: out[mi] = sum_jo vT[jo]^T @ WoF ---
            for mi in range(MC):
                ps = psum_s2.tile([P, 2, 512], f32, tag="s2")
                for half in range(NH):
                    for jo in range(JO):
                        nc.tensor.matmul(
                            ps[:, half, :],
                            lhsT=vt_sb[:, jo, ts(mi, P)],
                            rhs=wof_sb[:, half, jo, :],
                            start=(jo == 0),
                            stop=(jo == JO - 1),
                        )
                # drain each chunk as two half copies in parallel on DVE+ACT
                # (a merged 1024-wide copy is ~1.15us serial on one engine and
                # the last one gated the final store by ~1us)
                o_sb = o_pool.tile([P, D_MODEL], f16, tag="o_sb")
                eng_a, eng_b = (
                    (nc.vector.tensor_copy, nc.scalar.copy)
                    if mi % 2 == 0
                    else (nc.scalar.copy, nc.vector.tensor_copy)
                )
                eng_a(o_sb[:, :512], ps[:, 0, :])
                eng_b(o_sb[:, 512:], ps[:, 1, :])
                nc.sync.dma_start(out_d[ts(mi, P), :], o_sb[:])

    nc.finalize()
    return nc


def _get_nc():
    global _nc_cache
    if _nc_cache is None:
        _nc_cache = _build_bass()
    return _nc_cache


def kernel(**inputs) -> np.ndarray:
    global LAST_RESULTS
    from concourse.bass_utils import run_bass_kernel_spmd

    x = np.asarray(inputs["x"], dtype=np.float32).reshape(M_TOTAL, D_MODEL)
    wv = np.asarray(inputs["Wv"], dtype=np.float32)
    wo = np.asarray(inputs["Wo"], dtype=np.float32)

    # GQA fold (sum the 4 query-head row blocks per KV head), then fp16,
    # then the [p, (h, jo, n)] device layout.
    wof = wo.reshape(4, 4, 64, D_MODEL).sum(axis=1).reshape(NKV, D_MODEL)
    wof16 = np.ascontiguousarray(
        wof.astype(np.float16)
        .reshape(JO, P, NH, 512)
        .transpose(1, 2, 0, 3)          # -> [p, h, jo, n]
        .reshape(P, NH * JO * 512)
    )
    wv16 = np.ascontiguousarray(
        wv.astype(np.float16).reshape(KK, P, NKV).transpose(1, 0, 2).reshape(P, KK * NKV)
    )

    nc = _get_nc()
    in_maps = []
    for i in range(N_CORES):
        xr = x[i * M_CORE : (i + 1) * M_CORE]          # [512, 1024]
        xt = np.ascontiguousarray(
            xr.T.astype(np.float16)                     # [1024, 512] = [ko*128+p, m]
            .reshape(KK, P, M_CORE)
            .transpose(1, 0, 2)                         # -> [p, ko, m]
            .reshape(P, KK * M_CORE)
        )
        in_maps.append({"xt": xt, "wv": wv16, "wof": wof16})

    res = run_bass_kernel_spmd(
        nc,
        in_maps,
        list(range(N_CORES)),
        trace=TRACE,
        trace_cores=TRACE_CORES,
    )
    LAST_RESULTS = res
    out = np.concatenate([r["out"] for r in res.results], axis=0)
    return out.astype(np.float32).reshape(B, T, D_MODEL)
